# revision 6
# baseline (speedup 1.0000x reference)
"""Butterfly transform (12 layers, n=4096) on 8 Trainium2 NeuronCores.

Strategy
--------
The 12 butterfly layers split into two groups:
  * layers 0-5 (strides 2048..64) only mix the high 6 bits m = f>>6 of the
    feature index, independently for each low offset j = f % 64  ->  64
    independent 64x64 dense matrices H_j.
  * layers 6-11 (strides 32..1) only mix the low 6 bits j within each
    contiguous 64-block  ->  64 independent 64x64 dense matrices L_m.

Both groups map onto the 128x128 TensorEngine as per-tile matmuls:
  * stage H uses "j-grouped" feature tiles: tile u holds features
    f = 64*m + 2u + r  (p = 2m + r), so blockdiag(H_{2u}, H_{2u+1})
    interleaved is a dense 128x128 weight.
  * stage L uses natural contiguous 128-feature tiles: tile t holds
    f = 128t + p, weight = blockdiag(L_{2t}, L_{2t+1}).

Data flow per 128-row batch tile (x stays batch-major in HBM/SBUF, the PE
transpose produces the feature-major operand; the j->m regrouping between
the stages is free because it happens on the batch-major free dimension):
  DMA in -> PE transpose (f32, j-grouped cols) -> cast-evac fp16 ->
  matmul (data stationary, Wh moving) -> evac fp16 (strided dest) ->
  PE transpose (fp16, contiguous cols) -> evac fp16 ->
  matmul (Wl moving) -> evac f32 -> DMA out.

Weights are composed on the host from the twiddles (tiny: 196K params) by
running the butterfly layer groups over an identity matrix in float64.

Sharding: data-parallel over batch — 8192 rows / 8 cores = 1024 rows each;
weights replicated.
"""

import numpy as np

SIZE = 4096
LOG_N = 12
BATCH = 8192
N_CORES = 8
BPC = BATCH // N_CORES  # 1024 batch rows per core
P = 128
N_GROUPS = SIZE // P  # 32

# compute dtype for the matmul stages: "float16" or "float32"
COMPUTE_DT = "float16"


def _butterfly_layers(x, twiddles, layers):
    B, n = x.shape
    out = x
    for layer in layers:
        stride = 2 ** (LOG_N - layer - 1)
        n_blocks = n // (2 * stride)
        xr = out.reshape(B, n_blocks, 2, stride)
        top = xr[:, :, 0, :]
        bot = xr[:, :, 1, :]
        w = twiddles[layer].reshape(n_blocks, stride, 2, 2)
        new_top = w[None, :, :, 0, 0] * top + w[None, :, :, 0, 1] * bot
        new_bot = w[None, :, :, 1, 0] * top + w[None, :, :, 1, 1] * bot
        out = np.stack([new_top, new_bot], axis=2).reshape(B, n)
    return out


def _compose_weights(twiddles):
    """twiddles [12, 2048, 2, 2] f32 -> (wh, wl) each [128, 32*128].

    wh[:, u*128:(u+1)*128][p, p'] = H[f_u(p'), f_u(p)] with
    f_u(p) = 64*(p>>1) + 2u + (p&1); wl analogous for contiguous tiles.
    Layouts are pre-swizzled for a single contiguous DMA into SBUF
    [128 partitions, 4096] (partition = contraction index p).
    """
    tw = np.asarray(twiddles, dtype=np.float64)
    I = np.eye(SIZE, dtype=np.float64)
    # butterfly(I) rows are transformed basis vectors => butterfly(I) = M^T
    H_T = _butterfly_layers(I, tw, range(0, 6))  # H_T[f, f'] = H[f', f]
    L_T = _butterfly_layers(I, tw, range(6, 12))

    p = np.arange(P)
    wh = np.zeros((P, N_GROUPS * P), np.float64)
    for u in range(N_GROUPS):
        # group u holds features f = 32*p + u  (j-pair {u, u+32}, all m;
        # p = 2m + r with j = u + 32r) — a single-stride column selection,
        # required because matmul stationary APs allow only one free dim.
        f_idx = 32 * p + u
        # Wh_u[p, p'] = H[f(p'), f(p)] = H_T[f(p), f(p')]
        wh[:, u * P : (u + 1) * P] = H_T[np.ix_(f_idx, f_idx)]
    wl = np.zeros((P, N_GROUPS * P), np.float64)
    for t in range(N_GROUPS):
        blk = slice(P * t, P * (t + 1))
        # Wl_t[p', p''] = L[f''(p''), f'(p')] = L_T[f'(p'), f''(p'')]
        wl[:, t * P : (t + 1) * P] = L_T[blk, blk]
    np_dt = np.float16 if COMPUTE_DT == "float16" else np.float32
    return wh.astype(np_dt), wl.astype(np_dt)


_NC_CACHE = {}


def _build_nc():
    if "nc" in _NC_CACHE:
        return _NC_CACHE["nc"]

    import concourse.mybir as mybir
    from concourse import bacc
    from concourse.bass import ts
    from concourse.masks import make_identity
    from concourse.tile import TileContext

    f32 = mybir.dt.float32
    cdt = getattr(mybir.dt, COMPUTE_DT)

    nc = bacc.Bacc("TRN2", target_bir_lowering=False, debug=False)
    x_ext = nc.declare_dram_parameter("x", [BPC, SIZE], f32, isOutput=False)
    wh_ext = nc.declare_dram_parameter("wh", [P, N_GROUPS * P], cdt, isOutput=False)
    wl_ext = nc.declare_dram_parameter("wl", [P, N_GROUPS * P], cdt, isOutput=False)
    out_ext = nc.declare_dram_parameter("out", [BPC, SIZE], f32, isOutput=True)

    with TileContext(nc) as tc:
        with (
            tc.tile_pool(name="consts", bufs=1) as consts,
            tc.tile_pool(name="xin", bufs=2) as xin_pool,
            tc.tile_pool(name="xt", bufs=4) as xt_pool,
            tc.tile_pool(name="ybuf", bufs=2) as y_pool,
            tc.tile_pool(name="yt", bufs=4) as yt_pool,
            tc.tile_pool(name="obuf", bufs=2) as o_pool,
            tc.tile_pool(name="psum", bufs=2, space="PSUM") as psum_pool,
        ):
            ident32 = consts.tile([P, P], f32)
            make_identity(nc, ident32)
            if COMPUTE_DT != "float32":
                identc = consts.tile([P, P], cdt)
                make_identity(nc, identc)
            else:
                identc = ident32

            wh_sb = consts.tile([P, N_GROUPS * P], cdt)
            nc.sync.dma_start(wh_sb, wh_ext[:])
            wl_sb = consts.tile([P, N_GROUPS * P], cdt)
            nc.sync.dma_start(wl_sb, wl_ext[:])

            def copy(k, out, in_):
                # alternate engines to split evacuation work DVE/ACT
                if k % 2 == 0:
                    nc.vector.tensor_copy(out, in_)
                else:
                    nc.scalar.copy(out, in_)

            for i in range(BPC // P):
                x_sb = xin_pool.tile([P, SIZE], f32)
                nc.sync.dma_start(x_sb, x_ext[ts(i, P), :])
                # f = 32*k + u  ->  dims (k, u); group u = columns u::32
                xr = x_sb.rearrange("b (k u) -> b k u", k=P, u=N_GROUPS)
                y_sb = y_pool.tile([P, SIZE], cdt)
                yr = y_sb.rearrange("b (k u) -> b k u", k=P, u=N_GROUPS)

                for u in range(N_GROUPS):
                    pt1 = psum_pool.tile([P, P], f32, tag="t1")
                    nc.tensor.transpose(pt1, xr[:, :, u], ident32)
                    xt = xt_pool.tile([P, P], cdt, tag="xt")
                    copy(u, xt, pt1)
                    ph = psum_pool.tile([P, P], f32, tag="h")
                    nc.tensor.matmul(
                        ph, lhsT=xt, rhs=wh_sb[:, ts(u, P)], start=True, stop=True
                    )
                    copy(u + 1, yr[:, :, u], ph)

                o_sb = o_pool.tile([P, SIZE], f32)
                for t in range(N_GROUPS):
                    pt2 = psum_pool.tile([P, P], cdt, tag="t2")
                    nc.tensor.transpose(pt2, y_sb[:, ts(t, P)], identc)
                    yt = yt_pool.tile([P, P], cdt, tag="yt")
                    copy(t, yt, pt2)
                    pl = psum_pool.tile([P, P], f32, tag="l")
                    nc.tensor.matmul(
                        pl, lhsT=yt, rhs=wl_sb[:, ts(t, P)], start=True, stop=True
                    )
                    copy(t + 1, o_sb[:, ts(t, P)], pl)

                nc.sync.dma_start(out_ext[ts(i, P), :], o_sb)

    nc.compile()
    _NC_CACHE["nc"] = nc
    return nc


def _run(x, twiddles, **kwargs):
    from concourse.bass_utils import run_bass_kernel_spmd

    x = np.ascontiguousarray(np.asarray(x, dtype=np.float32))
    wh, wl = _compose_weights(twiddles)
    nc = _build_nc()
    in_maps = [
        {"x": x[i * BPC : (i + 1) * BPC], "wh": wh, "wl": wl}
        for i in range(N_CORES)
    ]
    res = run_bass_kernel_spmd(nc, in_maps, core_ids=list(range(N_CORES)), **kwargs)
    out = np.concatenate([res.results[i]["out"] for i in range(N_CORES)], axis=0)
    return out, res


def kernel(x, twiddles):
    out, _ = _run(x, twiddles)
    return out


# revision 10
# speedup vs baseline: 830.8867x; 830.8867x over previous
"""Butterfly transform (12 layers, n=4096) on 8 Trainium2 NeuronCores.

Strategy
--------
The 12 butterfly layers split into two groups:
  * layers 0-5 (strides 2048..64) only mix the high 6 bits m = f>>6 of the
    feature index, independently for each low offset j = f % 64  ->  64
    independent 64x64 dense matrices H_j.
  * layers 6-11 (strides 32..1) only mix the low 6 bits j within each
    contiguous 64-block  ->  64 independent 64x64 dense matrices L_m.

Both groups map onto the 128x128 TensorEngine as per-tile matmuls:
  * stage H uses "j-grouped" feature tiles: tile u holds features
    f = 64*m + 2u + r  (p = 2m + r), so blockdiag(H_{2u}, H_{2u+1})
    interleaved is a dense 128x128 weight.
  * stage L uses natural contiguous 128-feature tiles: tile t holds
    f = 128t + p, weight = blockdiag(L_{2t}, L_{2t+1}).

Data flow per 128-row batch tile (x stays batch-major in HBM/SBUF, the PE
transpose produces the feature-major operand; the j->m regrouping between
the stages is free because it happens on the batch-major free dimension):
  DMA in -> PE transpose (f32, j-grouped cols) -> cast-evac fp16 ->
  matmul (data stationary, Wh moving) -> evac fp16 (strided dest) ->
  PE transpose (fp16, contiguous cols) -> evac fp16 ->
  matmul (Wl moving) -> evac f32 -> DMA out.

Weights are composed on the host from the twiddles (tiny: 196K params) by
running the butterfly layer groups over an identity matrix in float64.

Sharding: data-parallel over batch — 8192 rows / 8 cores = 1024 rows each;
weights replicated.
"""

import numpy as np

SIZE = 4096
LOG_N = 12
BATCH = 8192
N_CORES = 8
BPC = BATCH // N_CORES  # 1024 batch rows per core
P = 128
N_GROUPS = SIZE // P  # 32

# compute dtype for the matmul stages: "float16" or "float32"
COMPUTE_DT = "float16"


def _butterfly_layers(x, twiddles, layers):
    B, n = x.shape
    out = x
    for layer in layers:
        stride = 2 ** (LOG_N - layer - 1)
        n_blocks = n // (2 * stride)
        xr = out.reshape(B, n_blocks, 2, stride)
        top = xr[:, :, 0, :]
        bot = xr[:, :, 1, :]
        w = twiddles[layer].reshape(n_blocks, stride, 2, 2)
        new_top = w[None, :, :, 0, 0] * top + w[None, :, :, 0, 1] * bot
        new_bot = w[None, :, :, 1, 0] * top + w[None, :, :, 1, 1] * bot
        out = np.stack([new_top, new_bot], axis=2).reshape(B, n)
    return out


def _compose_weights(twiddles):
    """twiddles [12, 2048, 2, 2] f32 -> (wh, wl) each [128, 32*128].

    wh[:, u*128:(u+1)*128][p, p'] = H[f_u(p'), f_u(p)] with
    f_u(p) = 64*(p>>1) + 2u + (p&1); wl analogous for contiguous tiles.
    Layouts are pre-swizzled for a single contiguous DMA into SBUF
    [128 partitions, 4096] (partition = contraction index p).
    """
    tw = np.asarray(twiddles, dtype=np.float64)
    I = np.eye(SIZE, dtype=np.float64)
    # butterfly(I) rows are transformed basis vectors => butterfly(I) = M^T
    H_T = _butterfly_layers(I, tw, range(0, 6))  # H_T[f, f'] = H[f', f]
    L_T = _butterfly_layers(I, tw, range(6, 12))

    p = np.arange(P)
    wh = np.zeros((P, N_GROUPS * P), np.float64)
    for u in range(N_GROUPS):
        # group u holds features f = 32*p + u  (j-pair {u, u+32}, all m;
        # p = 2m + r with j = u + 32r) — a single-stride column selection,
        # required because matmul stationary APs allow only one free dim.
        f_idx = 32 * p + u
        # Wh_u[p, p'] = H[f(p'), f(p)] = H_T[f(p), f(p')]
        wh[:, u * P : (u + 1) * P] = H_T[np.ix_(f_idx, f_idx)]
    wl = np.zeros((P, N_GROUPS * P), np.float64)
    for t in range(N_GROUPS):
        blk = slice(P * t, P * (t + 1))
        # Wl_t[p', p''] = L[f''(p''), f'(p')] = L_T[f'(p'), f''(p'')]
        wl[:, t * P : (t + 1) * P] = L_T[blk, blk]
    np_dt = np.float16 if COMPUTE_DT == "float16" else np.float32
    return wh.astype(np_dt), wl.astype(np_dt)


_NC_CACHE = {}


def _build_nc(repeat=1, noop=False):
    key = (repeat, noop, BPC)
    if key in _NC_CACHE:
        return _NC_CACHE[key]

    import concourse.mybir as mybir
    from concourse import bacc
    from concourse.bass import ts
    from concourse.masks import make_identity
    from concourse.tile import TileContext

    f32 = mybir.dt.float32
    cdt = getattr(mybir.dt, COMPUTE_DT)

    nc = bacc.Bacc("TRN2", target_bir_lowering=False, debug=False)
    x_ext = nc.declare_dram_parameter("x", [BPC, SIZE], f32, isOutput=False)
    wh_ext = nc.declare_dram_parameter("wh", [P, N_GROUPS * P], cdt, isOutput=False)
    wl_ext = nc.declare_dram_parameter("wl", [P, N_GROUPS * P], cdt, isOutput=False)
    out_ext = nc.declare_dram_parameter("out", [BPC, SIZE], f32, isOutput=True)

    if noop:
        # minimal kernel with identical I/O signature, for dispatch-overhead
        # measurement: copy one 128-row tile through SBUF
        with TileContext(nc) as tc:
            with tc.tile_pool(name="tiny", bufs=1) as pool:
                t = pool.tile([P, P], f32)
                nc.sync.dma_start(t, x_ext[0:P, 0:P])
                nc.sync.dma_start(out_ext[0:P, 0:P], t)
        nc.compile()
        _NC_CACHE[key] = nc
        return nc

    with TileContext(nc) as tc:
        with (
            tc.tile_pool(name="consts", bufs=1) as consts,
            tc.tile_pool(name="xin", bufs=2) as xin_pool,
            tc.tile_pool(name="xt", bufs=4) as xt_pool,
            tc.tile_pool(name="ybuf", bufs=2) as y_pool,
            tc.tile_pool(name="yt", bufs=4) as yt_pool,
            tc.tile_pool(name="obuf", bufs=2) as o_pool,
            tc.tile_pool(name="psum", bufs=2, space="PSUM") as psum_pool,
        ):
            ident32 = consts.tile([P, P], f32)
            make_identity(nc, ident32)
            if COMPUTE_DT != "float32":
                identc = consts.tile([P, P], cdt)
                make_identity(nc, identc)
            else:
                identc = ident32

            wh_sb = consts.tile([P, N_GROUPS * P], cdt)
            nc.sync.dma_start(wh_sb, wh_ext[:])
            wl_sb = consts.tile([P, N_GROUPS * P], cdt)
            nc.sync.dma_start(wl_sb, wl_ext[:])

            def copy(k, out, in_):
                # alternate engines to split evacuation work DVE/ACT
                if k % 2 == 0:
                    nc.vector.tensor_copy(out, in_)
                else:
                    nc.scalar.copy(out, in_)

            for i in [bi for _ in range(repeat) for bi in range(BPC // P)]:
                x_sb = xin_pool.tile([P, SIZE], f32)
                nc.sync.dma_start(x_sb, x_ext[ts(i, P), :])
                # f = 32*k + u  ->  dims (k, u); group u = columns u::32
                xr = x_sb.rearrange("b (k u) -> b k u", k=P, u=N_GROUPS)
                y_sb = y_pool.tile([P, SIZE], cdt)
                yr = y_sb.rearrange("b (k u) -> b k u", k=P, u=N_GROUPS)

                for u in range(N_GROUPS):
                    pt1 = psum_pool.tile([P, P], f32, tag="t1")
                    nc.tensor.transpose(pt1, xr[:, :, u], ident32)
                    xt = xt_pool.tile([P, P], cdt, tag="xt")
                    copy(u, xt, pt1)
                    ph = psum_pool.tile([P, P], f32, tag="h")
                    nc.tensor.matmul(
                        ph, lhsT=xt, rhs=wh_sb[:, ts(u, P)], start=True, stop=True
                    )
                    copy(u + 1, yr[:, :, u], ph)

                o_sb = o_pool.tile([P, SIZE], f32)
                for t in range(N_GROUPS):
                    pt2 = psum_pool.tile([P, P], cdt, tag="t2")
                    nc.tensor.transpose(pt2, y_sb[:, ts(t, P)], identc)
                    yt = yt_pool.tile([P, P], cdt, tag="yt")
                    copy(t, yt, pt2)
                    pl = psum_pool.tile([P, P], f32, tag="l")
                    nc.tensor.matmul(
                        pl, lhsT=yt, rhs=wl_sb[:, ts(t, P)], start=True, stop=True
                    )
                    copy(t + 1, o_sb[:, ts(t, P)], pl)

                nc.sync.dma_start(out_ext[ts(i, P), :], o_sb)

    nc.compile()
    _NC_CACHE[key] = nc
    return nc


def _run(x, twiddles, **kwargs):
    from concourse.bass_utils import run_bass_kernel_spmd

    x = np.ascontiguousarray(np.asarray(x, dtype=np.float32))
    wh, wl = _compose_weights(twiddles)
    nc = _build_nc()
    in_maps = [
        {"x": x[i * BPC : (i + 1) * BPC], "wh": wh, "wl": wl}
        for i in range(N_CORES)
    ]
    res = run_bass_kernel_spmd(nc, in_maps, core_ids=list(range(N_CORES)), **kwargs)
    out = np.concatenate([res.results[i]["out"] for i in range(N_CORES)], axis=0)
    return out, res


def kernel(x, twiddles):
    out, _ = _run(x, twiddles)
    return out
